# revision 37
# baseline (speedup 1.0000x reference)
"""Distributed GCN (5x GraphConv(add) + residual/ReLU + mean-pool + linear)
for 8 Trainium2 NeuronCores.

Sharding: nodes partitioned contiguously across cores (1280 nodes/core, padded
to 10240). Each core owns the edges whose *destination* lands in its shard.
Aggregation is computed as A@(x@Wr): project first (p = x@Wr, bf16 matmuls),
quantize p to fp8-e4m3, AllGather, gather p[src[e]] rows with SWDGE
dma_gather (512B rows, half the bf16 traffic), then reduce edge tiles onto
destination nodes with fp8 DoubleRow segment matmuls (2 edge-tiles per PE
instruction). Weights/x stay bf16: fp8 weight-quant error is correlated
across nodes and survives mean-pooling, while fp8-p error averages out over
~16 aggregated edges (rel err 7e-3 vs 5e-3 all-bf16).

Overlap: the first LT=2 edge-tiles of each block hold only local-source
edges, gathered from p_shard before the AllGather lands; root-term (x@Ws +
bias) PSUM groups are emitted one block ahead (psA ping-pong) so PE has
dependency-free work while gathers drain; constants stream critical-path
first (xt0+Wr[0] before everything); residual add + ReLU both on DVE (no
cross-engine hop on the xt critical path); p PSUM->fp8 casts and mean-pool
accumulation on the Activation engine. fp32 PSUM accumulation throughout.
"""

import numpy as np
import ml_dtypes

BF16 = ml_dtypes.bfloat16
F8 = ml_dtypes.float8_e4m3

N, E, D, OUT, G = 10000, 160000, 512, 128, 64
NCORES, P = 8, 128
NBLK = 10                     # 128-node blocks per core
NC_NODES = NBLK * P           # 1280
NPAD = NCORES * NC_NODES      # 10240
NLAYERS = 5
KD = D // P                   # 4 chunks of in-channels
LT = 2                        # leading edge-tiles per block reserved for
                              # local-source edges (gathered from p_shard
                              # before the AllGather completes)


def _wrap_idx(a):
    """[L] ints -> [128, L//16] int16 SWDGE index layout (16-partition wrap,
    replicated for the 8 Q7 cores)."""
    L = len(a)
    w = a.astype(np.int16).reshape(L // 16, 16).T
    return np.ascontiguousarray(np.tile(w, (8, 1)))


_ALPHA_CACHE = {}


def _layer_alphas(x, ss_, ds_, inputs):
    """Per-layer power-of-2 scales so fp8(alpha_l * x_l) never overflows
    e4m3 (+-240). Exact f32 forward pass with reduceat segment sums."""
    key = (float(x[0, :8].sum()), float(x[-1, :8].sum()))
    if key in _ALPHA_CACHE:
        return _ALPHA_CACHE[key]
    node_starts = np.searchsorted(ds_, np.arange(N + 1))
    cnts = np.diff(node_starts)
    nz = cnts > 0
    starts_nz = node_starts[:-1][nz]
    xs = x.astype(np.float32)
    alphas = []
    for l in range(NLAYERS):
        m = float(np.abs(xs).max())
        alphas.append(min(1.0, 2.0 ** np.floor(np.log2(100.0 / max(m, 1e-9)))))
        gath = xs[ss_]
        agg = np.zeros((N, D), np.float32)
        agg[nz] = np.add.reduceat(gath, starts_nz, axis=0)
        wr_l = np.asarray(inputs[f"Wr{l+1}"], np.float32)
        ws_l = np.asarray(inputs[f"Ws{l+1}"], np.float32)
        b_l = np.asarray(inputs[f"b{l+1}"], np.float32)
        pm = float(np.abs(xs @ wr_l).max())   # per-node p = x@Wr is fp8-cast
        assert pm < 200.0, f"fp8 p overflow risk at layer {l}: {pm}"
        val = xs + agg @ wr_l + b_l + xs @ ws_l
        xs = np.maximum(val, 0) if l < NLAYERS - 1 else val
    _ALPHA_CACHE[key] = alphas
    return alphas


def _prep(inputs):
    x = np.asarray(inputs["x"], np.float32)
    ei = np.asarray(inputs["edge_index"]).astype(np.int64)
    batch = np.asarray(inputs["batch"]).astype(np.int64)
    src, dst = ei[0], ei[1]

    order = np.argsort(dst, kind="stable")
    ds_, ss_ = dst[order], src[order]
    alphas = [1.0] * NLAYERS    # x/weights stay bf16 (fp8 weights fail the
                                # error gate: their quant error is correlated
                                # across nodes so pooling can't average it)
    starts = np.searchsorted(ds_, np.arange(0, NPAD + 1, P))
    counts = np.diff(starts)
    T_pad = max(2, int(np.ceil(counts.max() / P)))
    # remote region (T_pad - LT tiles) must hold every block's non-local edges
    nremote = []
    for gb in range(NPAD // P):
        c_ = gb // NBLK
        sl = ss_[starts[gb]:starts[gb + 1]]
        isloc = (sl >= c_ * NC_NODES) & (sl < (c_ + 1) * NC_NODES)
        nloc = min(int(isloc.sum()), LT * P)
        nremote.append(len(sl) - nloc)
    T_pad = max(T_pad, LT + int(np.ceil(max(nremote) / P)))
    T_pad += T_pad % 2          # even, for fp8 DoubleRow tile pairs
    L = T_pad * P

    xp = np.zeros((NPAD, D), np.float32)
    xp[:N] = x

    counts_g = np.bincount(batch, minlength=G)[:G]
    inv = (1.0 / np.maximum(counts_g, 1.0)).astype(np.float32)

    per_core = []
    for c in range(NCORES):
        idx_blocks = []
        loc_blocks = []
        oh_flat = np.zeros((P, NBLK * T_pad, P), F8)
        goh = np.zeros((P, NBLK * G), BF16)
        for b in range(NBLK):
            gb = c * NBLK + b
            lo = gb * P
            s0, s1 = int(starts[gb]), int(starts[gb + 1])
            n = s1 - s0
            sl = ss_[s0:s1]
            dl = ds_[s0:s1] - lo
            isloc = (sl >= c * NC_NODES) & (sl < (c + 1) * NC_NODES)
            loc = np.where(isloc)[0]
            rem = np.where(~isloc)[0]
            nl = min(len(loc), LT * P)
            rest = np.concatenate([rem, loc[nl:]])
            assert len(rest) <= (T_pad - LT) * P, "remote edge region overflow"
            srcs = np.zeros(L, np.int64)
            srcs[:nl] = sl[loc[:nl]] - c * NC_NODES     # p_shard-relative
            srcs[LT * P:LT * P + len(rest)] = sl[rest]  # p_full (global)
            oh = np.zeros((L, P), F8)
            oh[np.arange(nl), dl[loc[:nl]]] = 1
            oh[LT * P + np.arange(len(rest)), dl[rest]] = 1
            idx_blocks.append(_wrap_idx(srcs))
            loc_blocks.append(_wrap_idx(srcs[:LT * P]))
            oh_flat[:, b * T_pad:(b + 1) * T_pad, :] = (
                oh.reshape(T_pad, P, P).transpose(1, 0, 2))
            nodes = lo + np.arange(P)
            valid = nodes < N
            goh[valid, b * G + batch[nodes[valid]]] = 1

        shard = xp[c * NC_NODES:(c + 1) * NC_NODES].astype(BF16)
        xt0 = np.ascontiguousarray(
            shard.T.reshape(KD, P, NC_NODES).transpose(1, 0, 2))
        per_core.append(dict(
            x_shard=np.ascontiguousarray(shard),
            xt0=xt0,
            ohot=oh_flat,
            idxe=np.ascontiguousarray(np.concatenate(idx_blocks, axis=1)),
            idxl=np.ascontiguousarray(np.concatenate(loc_blocks, axis=1)),
            goh=goh,
        ))

    wr = np.zeros((P, NLAYERS, KD, D), BF16)
    ws = np.zeros((P, NLAYERS, KD, D), BF16)
    bias = np.zeros((P, NLAYERS, D), BF16)
    for l in range(NLAYERS):
        wr[:, l] = np.asarray(inputs[f"Wr{l+1}"], np.float32).reshape(
            KD, P, D).transpose(1, 0, 2).astype(BF16)
        ws[:, l] = np.asarray(inputs[f"Ws{l+1}"], np.float32).reshape(
            KD, P, D).transpose(1, 0, 2).astype(BF16)
        bias[0, l] = np.asarray(inputs[f"b{l+1}"], np.float32).astype(BF16)
    ones_e0 = np.zeros((P, P), BF16)
    ones_e0[0, :] = 1
    wlin = np.ascontiguousarray(
        np.asarray(inputs["Wlin"], np.float32).reshape(KD, P, OUT)
        .transpose(1, 0, 2).astype(BF16))
    blin = np.tile(np.asarray(inputs["blin"], np.float32).reshape(OUT, 1),
                   (1, 1)).astype(np.float32)
    shared = dict(
        wr=wr, ws=ws, bias=bias, ones=ones_e0, wlin=wlin, blin=blin,
        invt=np.ascontiguousarray(np.tile(inv, (P, KD)).astype(np.float32)),
        ident=np.eye(P, dtype=BF16),
    )
    return per_core, shared, T_pad, alphas


def _unwrap(w, L):
    """inverse of _wrap_idx: [128, L//16] -> [L]"""
    return np.ascontiguousarray(w[:16].T).reshape(-1)[:L].astype(np.int64)


def emulate(inputs):
    """Numpy emulation of the exact device dataflow (bf16 casts included).
    Validates all host-side index/one-hot bookkeeping."""
    per_core, shared, T_pad, alphas = _prep(inputs)
    L = T_pad * P
    f32 = np.float32

    xs = [pc["x_shard"].astype(f32) for pc in per_core]       # [1280, 512]
    for l in range(NLAYERS):
        wr_l = np.concatenate([shared["wr"][:, l, k, :] for k in range(KD)],
                              axis=0).astype(f32)             # [512, 512]
        ws_l = np.concatenate([shared["ws"][:, l, k, :] for k in range(KD)],
                              axis=0).astype(f32)
        b_l = shared["bias"][0, l].astype(f32)
        xq = xs
        # p = xq @ Wr, cast fp8, "AllGather"
        p_full = np.concatenate(
            [(xq[c] @ wr_l).astype(F8).astype(f32) for c in range(NCORES)],
            axis=0)                                           # [10240, 512]
        new_xs = []
        for c in range(NCORES):
            nx = np.zeros((NC_NODES, D), f32)
            for b in range(NBLK):
                idx = _unwrap(
                    per_core[c]["idxe"][:, b * (L // 16):(b + 1) * (L // 16)], L)
                idx = idx.copy()
                idx[:LT * P] += c * NC_NODES       # local tiles: shard-relative
                gath = p_full[idx]                            # [L, 512] fp8 exact
                acc = np.zeros((P, D), f32)
                for t in range(T_pad):
                    oh = per_core[c]["ohot"][
                        :, b * T_pad + t, :].astype(f32)      # [128e, 128d]
                    acc += oh.T @ gath[t * P:(t + 1) * P]
                blk = xs[c][b * P:(b + 1) * P]
                acc += xq[c][b * P:(b + 1) * P] @ ws_l + b_l
                val = (acc.astype(f32) + blk)
                if l < NLAYERS - 1:
                    val = np.maximum(val, 0)
                nx[b * P:(b + 1) * P] = val.astype(BF16).astype(f32)
            new_xs.append(nx)
        xs = new_xs
    # pooling
    pooled_T = np.zeros((D, G), f32)
    for c in range(NCORES):
        goh = per_core[c]["goh"].astype(f32)
        for b in range(NBLK):
            blk = xs[c][b * P:(b + 1) * P].astype(BF16).astype(f32)
            for j in range(KD):
                pooled_T[j * P:(j + 1) * P] += (
                    blk[:, j * P:(j + 1) * P].T @ goh[:, b * G:(b + 1) * G])
    inv = shared["invt"][0, :G].astype(f32)
    pooled_T = (pooled_T * inv[None, :]).astype(BF16).astype(f32)
    wlin = np.concatenate([shared["wlin"][:, k, :] for k in range(KD)],
                          axis=0).astype(f32)                 # [512, 128]
    out_T = wlin.T @ pooled_T + shared["blin"][:, :1]         # [128, 64]
    return np.ascontiguousarray(out_T.T).astype(np.float32)


def _build(T_pad, alphas=None, enable_asserts=False):
    if alphas is None:
        alphas = [1.0] * NLAYERS
    import os
    n_layers = int(os.environ.get("GCN_LAYERS", NLAYERS))
    no_gather = bool(int(os.environ.get("GCN_NO_GATHER", "0")))
    no_cc = bool(int(os.environ.get("GCN_NO_CC", "0")))
    bP, bA, bT = (int(v) for v in os.environ.get("GCN_BANKS", "2,3,2").split(","))
    gbufs = int(os.environ.get("GCN_GBUFS", "3"))
    gsplit = int(os.environ.get("GCN_GSPLIT", "2"))
    no_tr = bool(int(os.environ.get("GCN_NO_TR", "0")))      # timing expts only
    import concourse.bass as bass
    import concourse.mybir as mybir
    import concourse.tile as tile
    from concourse import bacc

    F32 = mybir.dt.float32
    BF = mybir.dt.bfloat16
    FP8 = mybir.dt.float8e4
    I16 = mybir.dt.int16
    DROW = mybir.MatmulPerfMode.DoubleRow
    ADD = mybir.AluOpType.add
    MUL = mybir.AluOpType.mult
    L = T_pad * P
    RG = [list(range(NCORES))]

    nc = bacc.Bacc("TRN2", target_bir_lowering=False, debug=False,
                   enable_asserts=enable_asserts, num_devices=NCORES)

    # per-core inputs
    x_d = nc.dram_tensor("x_shard", [NC_NODES, D], BF, kind="ExternalInput")
    xt0_d = nc.dram_tensor("xt0", [P, KD, NC_NODES], BF, kind="ExternalInput")
    oh_d = nc.dram_tensor("ohot", [P, NBLK * T_pad, P], FP8, kind="ExternalInput")
    idxe_d = nc.dram_tensor("idxe", [P, NBLK * (L // 16)], I16, kind="ExternalInput")
    idxl_d = nc.dram_tensor("idxl", [P, NBLK * LT * 8], I16, kind="ExternalInput")
    goh_d = nc.dram_tensor("goh", [P, NBLK * G], BF, kind="ExternalInput")
    # shared inputs
    wr_d = nc.dram_tensor("wr", [P, NLAYERS, KD, D], BF, kind="ExternalInput")
    ws_d = nc.dram_tensor("ws", [P, NLAYERS, KD, D], BF, kind="ExternalInput")
    bias_d = nc.dram_tensor("bias", [P, NLAYERS, D], BF, kind="ExternalInput")
    ones_d = nc.dram_tensor("ones", [P, P], BF, kind="ExternalInput")
    wlin_d = nc.dram_tensor("wlin", [P, KD, OUT], BF, kind="ExternalInput")
    blin_d = nc.dram_tensor("blin", [OUT, 1], F32, kind="ExternalInput")
    invt_d = nc.dram_tensor("invt", [P, KD * G], F32, kind="ExternalInput")
    ident_d = nc.dram_tensor("ident", [P, P], BF, kind="ExternalInput")
    # internal DRAM (double-buffered by layer parity so the AllGather for
    # layer l+1 never WAR-depends on layer l's gathers)
    p_shard = [nc.dram_tensor(f"p_shard{i}", [NC_NODES, D], FP8) for i in (0, 1)]
    p_full = [nc.dram_tensor(f"p_full{i}", [NPAD, D], FP8, addr_space="Shared")
              for i in (0, 1)]
    pool_in = nc.dram_tensor("pool_in", [P, KD * G], F32)
    pool_out = nc.dram_tensor("pool_out", [P, KD * G], F32, addr_space="Shared")
    # output
    out_d = nc.dram_tensor("out_t", [OUT, G], F32, kind="ExternalOutput")

    with tile.TileContext(nc) as tc:
        with (
            tc.tile_pool(name="const", bufs=1) as const,
            tc.tile_pool(name="xs", bufs=2) as xpool,
            tc.tile_pool(name="xt", bufs=2) as xtpool,
            tc.tile_pool(name="gath", bufs=gbufs) as gpool,
            tc.tile_pool(name="small", bufs=int(os.environ.get("GCN_SBUFS", "4"))) as spool,
            tc.tile_pool(name="psP", bufs=bP, space="PSUM") as psP,
            tc.tile_pool(name="psA", bufs=bA, space="PSUM") as psA,
            tc.tile_pool(name="psS", bufs=1, space="PSUM") as psS,
            tc.tile_pool(name="psT", bufs=bT, space="PSUM") as psT,
        ):
            # ---- constants to SBUF, critical-path-first: the prologue
            # projection needs only xt0 + Wr[0]; everything else streams in
            # behind it so PE starts ~5us in instead of ~37us
            xt_cur = xtpool.tile([P, KD, NC_NODES], BF, tag="xt")
            nc.sync.dma_start(xt_cur[:], xt0_d[:])
            wr_sb = const.tile([P, NLAYERS, KD, D], BF, tag="wr")
            nc.sync.dma_start(wr_sb[:, 0], wr_d[:, 0])
            ws_sb = const.tile([P, NLAYERS, KD, D], BF, tag="ws")
            nc.sync.dma_start(ws_sb[:, 0], ws_d[:, 0])
            bias_sb = const.tile([P, NLAYERS, D], BF, tag="bias")
            nc.sync.dma_start(bias_sb[:], bias_d[:])
            ones_sb = const.tile([P, P], BF, tag="ones")
            nc.sync.dma_start(ones_sb[:], ones_d[:])
            xs_cur = xpool.tile([P, NBLK, D], BF, tag="xs")
            nc.sync.dma_start(xs_cur[:], x_d.ap().rearrange("(b p) d -> p b d", p=P))
            idxe_sb = const.tile([P, NBLK * (L // 16)], I16, tag="idxe")
            oh_sb = const.tile([P, NBLK * T_pad, P], FP8, tag="oh")
            for cb in range(NBLK):
                nc.sync.dma_start(
                    idxe_sb[:, cb * (L // 16):(cb + 1) * (L // 16)],
                    idxe_d[:, cb * (L // 16):(cb + 1) * (L // 16)])
                nc.sync.dma_start(
                    oh_sb[:, cb * T_pad:(cb + 1) * T_pad, :],
                    oh_d[:, cb * T_pad:(cb + 1) * T_pad, :])
            ident_sb = const.tile([P, P], BF, tag="ident")
            nc.sync.dma_start(ident_sb[:], ident_d[:])
            # later-layer weights + pooling constants load from inside the
            # layer loop (below) so layer 0's stores/gathers aren't queued
            # behind 4.5MB of not-yet-needed constants
            goh_sb = const.tile([P, NBLK * G], BF, tag="goh")
            wlin_sb = const.tile([P, KD, OUT], BF, tag="wlin")
            blin_sb = const.tile([OUT, 1], F32, tag="blin")
            invt_sb = const.tile([P, KD * G], F32, tag="invt")

            def deferred_loads(l_, b_):
                if l_ == 0 and b_ in (3, 5):
                    for wl in (1, 2) if b_ == 3 else (3, 4):
                        nc.sync.dma_start(wr_sb[:, wl], wr_d[:, wl])
                        nc.sync.dma_start(ws_sb[:, wl], ws_d[:, wl])
                elif l_ == 2 and b_ == 0:
                    nc.sync.dma_start(goh_sb[:], goh_d[:])
                elif l_ == 3 and b_ == 0:
                    nc.sync.dma_start(wlin_sb[:], wlin_d[:])
                    nc.sync.dma_start(blin_sb[:], blin_d[:])
                    nc.sync.dma_start(invt_sb[:], invt_d[:])

            def emit_p_block(xt_src, layer, m, pbuf):
                """p[l=layer] block m = x_l[block m] @ Wr_l, into p_shard[pbuf]."""
                pps = psP.tile([P, D], F32, tag="pps", name=f"pps_{layer}_{m}")
                for k in range(KD):
                    nc.tensor.matmul(
                        pps[:],
                        lhsT=xt_src[:, k, m * P:(m + 1) * P],
                        rhs=wr_sb[:, layer, k, :],
                        start=(k == 0), stop=(k == KD - 1))
                p_sb = spool.tile([P, D], FP8, tag="psb", name=f"psb_{layer}_{m}")
                nc.scalar.activation(
                    p_sb[:], pps[:], func=mybir.ActivationFunctionType.Copy)
                nc.sync.dma_start(
                    p_shard[pbuf][m * P:(m + 1) * P, :], p_sb[:])

            def emit_ag(pbuf):
                if no_cc:
                    nc.sync.dma_start(
                        p_full[pbuf][:NC_NODES, :], p_shard[pbuf][:])
                else:
                    nc.gpsimd.collective_compute(
                        "AllGather", mybir.AluOpType.bypass, replica_groups=RG,
                        ins=[p_shard[pbuf][:]], outs=[p_full[pbuf][:]])

            def emit_wsbias(l_, xt_src, b_):
                """root-term matmuls for block b_; queued one block ahead so
                PE has dependency-free work while gathers/AllGather drain"""
                aps = psA.tile([P, D], F32, tag="aps")
                for k in range(KD):
                    nc.tensor.matmul(
                        aps[:],
                        lhsT=xt_src[:, k, b_ * P:(b_ + 1) * P],
                        rhs=ws_sb[:, l_, k, :],
                        start=(k == 0), stop=False)
                nc.tensor.matmul(
                    aps[:], lhsT=ones_sb[:], rhs=bias_sb[:, l_, :],
                    start=False, stop=False)
                return aps

            # prologue: projection for layer 0
            for m in range(NBLK):
                emit_p_block(xt_cur, 0, m, 0)
            emit_ag(0)
            wsdepth = int(os.environ.get("GCN_WSDEPTH", "2"))
            pend = [emit_wsbias(0, xt_cur, b_) for b_ in range(wsdepth)]

            pool_acc = const.tile([P, KD * G], F32, tag="pool_acc")
            for l in range(n_layers):
                pbuf = l % 2
                xs_next = xpool.tile([P, NBLK, D], BF, tag="xs")
                last = l == NLAYERS - 1
                if not last:
                    xt_next = xtpool.tile([P, KD, NC_NODES], BF, tag="xt")
                for b in range(NBLK):
                    deferred_loads(l, b)
                    g = gpool.tile([P, T_pad, D], FP8, tag="g")
                    if no_gather:
                        nc.vector.memset(g[:], 0)
                    else:
                        # tiles [0, LT): local-source edges gathered from
                        # p_shard -- no AllGather dependency, so their segment
                        # matmuls overlap the collective. Remaining tiles read
                        # p_full in even-sized chunks (DoubleRow pairs never
                        # span chunks)
                        col0 = b * (L // 16)
                        nc.gpsimd.dma_gather(
                            g[:, 0:LT, :], p_shard[pbuf][:],
                            idxe_sb[:, col0:col0 + LT * 8],
                            LT * P, LT * P, D, single_packet=False)
                        rem = T_pad - LT
                        if gsplit == 2 and rem > 6:
                            # uneven split: small first chunk releases segs
                            # sooner at the same Pool desc-gen call count
                            rf = int(os.environ.get("GCN_RFIRST", "8"))
                            bounds = [LT, LT + rf, T_pad]
                        else:
                            nsp = min(gsplit, max(1, rem // 2))
                            th = -2 * (-rem // (2 * nsp))
                            bounds = list(range(LT, T_pad, th)) + [T_pad]
                        for s0, s1 in zip(bounds[:-1], bounds[1:]):
                            nc.gpsimd.dma_gather(
                                g[:, s0:s1, :], p_full[pbuf][:],
                                idxe_sb[:, col0 + s0 * 8:col0 + s1 * 8],
                                (s1 - s0) * P, (s1 - s0) * P, D,
                                single_packet=False)
                    aps = pend.pop(0)
                    for t in range(0, T_pad, 2):
                        nc.tensor.matmul(
                            aps[:],
                            lhsT=oh_sb[:, b * T_pad + t:b * T_pad + t + 2, :],
                            rhs=g[:, t:t + 2, :],
                            start=False,
                            stop=(t + 2 >= T_pad),
                            perf_mode=DROW)
                    if last:
                        nc.vector.tensor_tensor(
                            xs_next[:, b, :], aps[:], xs_cur[:, b, :], op=ADD)
                        # pooling partials for this block, interleaved so they
                        # hide under later blocks' gathers; accumulate in SBUF
                        # so only one PSUM bank cycles here
                        for j in range(KD):
                            pps2 = psS.tile([P, G], F32, tag="pool")
                            nc.tensor.matmul(
                                pps2[:],
                                lhsT=xs_next[:, b, j * P:(j + 1) * P],
                                rhs=goh_sb[:, b * G:(b + 1) * G],
                                start=True, stop=True)
                            if b == 0:
                                nc.scalar.activation(
                                    pool_acc[:, j * G:(j + 1) * G], pps2[:],
                                    func=mybir.ActivationFunctionType.Copy)
                            else:
                                nc.vector.tensor_tensor(
                                    pool_acc[:, j * G:(j + 1) * G],
                                    pool_acc[:, j * G:(j + 1) * G], pps2[:],
                                    op=ADD)
                    else:
                        t1 = spool.tile([P, D], BF, tag="t1")
                        nc.vector.tensor_tensor(
                            t1[:], aps[:], xs_cur[:, b, :], op=ADD)
                        nc.vector.tensor_scalar_max(
                            xs_next[:, b, :], t1[:], 0.0)
                        # transpose new block into xt_next (channel-major)
                        if no_tr:
                            nc.vector.tensor_copy(
                                xt_next[:, :, b * P:(b + 1) * P],
                                xs_next[:, b, :].rearrange(
                                    "p (j q) -> p j q", j=KD)[:, :, :P])
                        else:
                            for j in range(KD):
                                trps = psT.tile([P, P], BF, tag="tr")
                                nc.tensor.transpose(
                                    trps[:], xs_next[:, b, j * P:(j + 1) * P],
                                    ident_sb[:])
                                nc.vector.tensor_copy(
                                    xt_next[:, j, b * P:(b + 1) * P], trps[:])
                        # pipelined projection for layer l+1, block b
                        emit_p_block(xt_next, l + 1, b, 1 - pbuf)
                    nb = b + wsdepth
                    if nb < NBLK:
                        pend.append(emit_wsbias(l, xt_cur, nb))
                    elif b == NBLK - 1 and not last:
                        # burst the next layer's first root-term groups HERE,
                        # after the last proj: this is the only queue position
                        # whose work can fill the AllGather+gather stall
                        for b_ in range(wsdepth):
                            pend.append(emit_wsbias(l + 1, xt_next, b_))
                if not last:
                    emit_ag(1 - pbuf)
                    xt_cur = xt_next
                xs_cur = xs_next

            # ---- pooling partials were accumulated inside the last layer's
            # block loop (one PSUM bank per 128-channel chunk)
            nc.sync.dma_start(pool_in[:], pool_acc[:])
            if no_cc:
                nc.sync.dma_start(pool_out[:], pool_acc[:])
            else:
                nc.gpsimd.collective_compute(
                    "AllReduce", ADD, replica_groups=RG,
                    ins=[pool_in[:]], outs=[pool_out[:]])
            pool2 = spool.tile([P, KD * G], F32, tag="pool2")
            nc.sync.dma_start(pool2[:], pool_out[:])
            poolbf = spool.tile([P, KD * G], BF, tag="poolbf")
            nc.vector.tensor_tensor(poolbf[:], pool2[:], invt_sb[:], op=MUL)
            fin_ps = psS.tile([P, G], F32, tag="pool", name="fin_ps")
            for k in range(KD):
                nc.tensor.matmul(
                    fin_ps[:], lhsT=wlin_sb[:, k, :],
                    rhs=poolbf[:, k * G:(k + 1) * G],
                    start=(k == 0), stop=(k == KD - 1))
            fin_sb = spool.tile([OUT, G], F32, tag="fin_sb")
            nc.vector.tensor_tensor(
                fin_sb[:], fin_ps[:], blin_sb[:, :1].to_broadcast([OUT, G]),
                op=ADD)
            nc.sync.dma_start(out_d[:], fin_sb[:])

    nc.compile()
    return nc


def kernel(**inputs):
    import os
    from concourse.bass_utils import run_bass_kernel_spmd

    per_core, shared, T_pad, alphas = _prep(inputs)
    nc = _build(T_pad, alphas)
    in_maps = [{**pc, **shared} for pc in per_core]
    trace = bool(int(os.environ.get("GCN_TRACE", "0")))
    res = run_bass_kernel_spmd(nc, in_maps, core_ids=list(range(NCORES)),
                               trace=trace)
    if trace:
        print(f"HW exec time: {res.exec_time_ns} ns")
        if res.instructions_and_trace is not None:
            print("trace:", res.instructions_and_trace[1])
    out_t = res.results[0]["out_t"]
    return np.ascontiguousarray(out_t.T).astype(np.float32)



# revision 40
# speedup vs baseline: 1.0022x; 1.0022x over previous
"""Distributed GCN (5x GraphConv(add) + residual/ReLU + mean-pool + linear)
for 8 Trainium2 NeuronCores.

Sharding: nodes partitioned contiguously across cores (1280 nodes/core, padded
to 10240). Each core owns the edges whose *destination* lands in its shard.
Aggregation is computed as A@(x@Wr): project first (p = x@Wr, bf16 matmuls),
quantize p to fp8-e4m3, AllGather, gather p[src[e]] rows with SWDGE
dma_gather (512B rows, half the bf16 traffic), then reduce edge tiles onto
destination nodes with fp8 DoubleRow segment matmuls (2 edge-tiles per PE
instruction). Weights/x stay bf16: fp8 weight-quant error is correlated
across nodes and survives mean-pooling, while fp8-p error averages out over
~16 aggregated edges (rel err 7e-3 vs 5e-3 all-bf16).

Overlap: the first LT=2 edge-tiles of each block hold only local-source
edges, gathered from p_shard before the AllGather lands; root-term (x@Ws +
bias) PSUM groups are emitted one block ahead (psA ping-pong) so PE has
dependency-free work while gathers drain; constants stream critical-path
first (xt0+Wr[0] before everything); residual add + ReLU both on DVE (no
cross-engine hop on the xt critical path); p PSUM->fp8 casts and mean-pool
accumulation on the Activation engine. fp32 PSUM accumulation throughout.
"""

import numpy as np
import ml_dtypes

BF16 = ml_dtypes.bfloat16
F8 = ml_dtypes.float8_e4m3

N, E, D, OUT, G = 10000, 160000, 512, 128, 64
NCORES, P = 8, 128
NBLK = 10                     # 128-node blocks per core
NC_NODES = NBLK * P           # 1280
NPAD = NCORES * NC_NODES      # 10240
NLAYERS = 5
KD = D // P                   # 4 chunks of in-channels
LT = 2                        # leading edge-tiles per block reserved for
                              # local-source edges (gathered from p_shard
                              # before the AllGather completes)


def _wrap_idx(a):
    """[L] ints -> [128, L//16] int16 SWDGE index layout (16-partition wrap,
    replicated for the 8 Q7 cores)."""
    L = len(a)
    w = a.astype(np.int16).reshape(L // 16, 16).T
    return np.ascontiguousarray(np.tile(w, (8, 1)))


_ALPHA_CACHE = {}


def _layer_alphas(x, ss_, ds_, inputs):
    """Per-layer power-of-2 scales so fp8(alpha_l * x_l) never overflows
    e4m3 (+-240). Exact f32 forward pass with reduceat segment sums."""
    key = (float(x[0, :8].sum()), float(x[-1, :8].sum()))
    if key in _ALPHA_CACHE:
        return _ALPHA_CACHE[key]
    node_starts = np.searchsorted(ds_, np.arange(N + 1))
    cnts = np.diff(node_starts)
    nz = cnts > 0
    starts_nz = node_starts[:-1][nz]
    xs = x.astype(np.float32)
    alphas = []
    for l in range(NLAYERS):
        m = float(np.abs(xs).max())
        alphas.append(min(1.0, 2.0 ** np.floor(np.log2(100.0 / max(m, 1e-9)))))
        gath = xs[ss_]
        agg = np.zeros((N, D), np.float32)
        agg[nz] = np.add.reduceat(gath, starts_nz, axis=0)
        wr_l = np.asarray(inputs[f"Wr{l+1}"], np.float32)
        ws_l = np.asarray(inputs[f"Ws{l+1}"], np.float32)
        b_l = np.asarray(inputs[f"b{l+1}"], np.float32)
        pm = float(np.abs(xs @ wr_l).max())   # per-node p = x@Wr is fp8-cast
        assert pm < 200.0, f"fp8 p overflow risk at layer {l}: {pm}"
        val = xs + agg @ wr_l + b_l + xs @ ws_l
        xs = np.maximum(val, 0) if l < NLAYERS - 1 else val
    _ALPHA_CACHE[key] = alphas
    return alphas


def _prep(inputs):
    x = np.asarray(inputs["x"], np.float32)
    ei = np.asarray(inputs["edge_index"]).astype(np.int64)
    batch = np.asarray(inputs["batch"]).astype(np.int64)
    src, dst = ei[0], ei[1]

    order = np.argsort(dst, kind="stable")
    ds_, ss_ = dst[order], src[order]
    alphas = [1.0] * NLAYERS    # x/weights stay bf16 (fp8 weights fail the
                                # error gate: their quant error is correlated
                                # across nodes so pooling can't average it)
    starts = np.searchsorted(ds_, np.arange(0, NPAD + 1, P))
    counts = np.diff(starts)
    T_pad = max(2, int(np.ceil(counts.max() / P)))
    # remote region (T_pad - LT tiles) must hold every block's non-local edges
    nremote = []
    for gb in range(NPAD // P):
        c_ = gb // NBLK
        sl = ss_[starts[gb]:starts[gb + 1]]
        isloc = (sl >= c_ * NC_NODES) & (sl < (c_ + 1) * NC_NODES)
        nloc = min(int(isloc.sum()), LT * P)
        nremote.append(len(sl) - nloc)
    T_pad = max(T_pad, LT + int(np.ceil(max(nremote) / P)))
    T_pad += T_pad % 2          # even, for fp8 DoubleRow tile pairs
    L = T_pad * P

    xp = np.zeros((NPAD, D), np.float32)
    xp[:N] = x

    counts_g = np.bincount(batch, minlength=G)[:G]
    inv = (1.0 / np.maximum(counts_g, 1.0)).astype(np.float32)

    per_core = []
    for c in range(NCORES):
        idx_blocks = []
        loc_blocks = []
        oh_flat = np.zeros((P, NBLK * T_pad, P), F8)
        goh = np.zeros((P, NBLK * G), BF16)
        for b in range(NBLK):
            gb = c * NBLK + b
            lo = gb * P
            s0, s1 = int(starts[gb]), int(starts[gb + 1])
            n = s1 - s0
            sl = ss_[s0:s1]
            dl = ds_[s0:s1] - lo
            isloc = (sl >= c * NC_NODES) & (sl < (c + 1) * NC_NODES)
            loc = np.where(isloc)[0]
            rem = np.where(~isloc)[0]
            nl = min(len(loc), LT * P)
            rest = np.concatenate([rem, loc[nl:]])
            assert len(rest) <= (T_pad - LT) * P, "remote edge region overflow"
            srcs = np.zeros(L, np.int64)
            srcs[:nl] = sl[loc[:nl]] - c * NC_NODES     # p_shard-relative
            srcs[LT * P:LT * P + len(rest)] = sl[rest]  # p_full (global)
            oh = np.zeros((L, P), F8)
            oh[np.arange(nl), dl[loc[:nl]]] = 1
            oh[LT * P + np.arange(len(rest)), dl[rest]] = 1
            idx_blocks.append(_wrap_idx(srcs))
            loc_blocks.append(_wrap_idx(srcs[:LT * P]))
            oh_flat[:, b * T_pad:(b + 1) * T_pad, :] = (
                oh.reshape(T_pad, P, P).transpose(1, 0, 2))
            nodes = lo + np.arange(P)
            valid = nodes < N
            goh[valid, b * G + batch[nodes[valid]]] = 1

        shard = xp[c * NC_NODES:(c + 1) * NC_NODES].astype(BF16)
        xt0 = np.ascontiguousarray(
            shard.T.reshape(KD, P, NC_NODES).transpose(1, 0, 2))
        per_core.append(dict(
            x_shard=np.ascontiguousarray(shard),
            xt0=xt0,
            ohot=oh_flat,
            idxe=np.ascontiguousarray(np.concatenate(idx_blocks, axis=1)),
            idxl=np.ascontiguousarray(np.concatenate(loc_blocks, axis=1)),
            goh=goh,
        ))

    wr = np.zeros((P, NLAYERS, KD, D), BF16)
    ws = np.zeros((P, NLAYERS, KD, D), BF16)
    bias = np.zeros((P, NLAYERS, D), BF16)
    for l in range(NLAYERS):
        wr[:, l] = np.asarray(inputs[f"Wr{l+1}"], np.float32).reshape(
            KD, P, D).transpose(1, 0, 2).astype(BF16)
        ws[:, l] = np.asarray(inputs[f"Ws{l+1}"], np.float32).reshape(
            KD, P, D).transpose(1, 0, 2).astype(BF16)
        bias[0, l] = np.asarray(inputs[f"b{l+1}"], np.float32).astype(BF16)
    ones_e0 = np.zeros((P, P), BF16)
    ones_e0[0, :] = 1
    wlin = np.ascontiguousarray(
        np.asarray(inputs["Wlin"], np.float32).reshape(KD, P, OUT)
        .transpose(1, 0, 2).astype(BF16))
    blin = np.tile(np.asarray(inputs["blin"], np.float32).reshape(OUT, 1),
                   (1, 1)).astype(np.float32)
    shared = dict(
        wr=wr, ws=ws, bias=bias, ones=ones_e0, wlin=wlin, blin=blin,
        invt=np.ascontiguousarray(np.tile(inv, (P, KD)).astype(np.float32)),
        ident=np.eye(P, dtype=BF16),
    )
    return per_core, shared, T_pad, alphas


def _unwrap(w, L):
    """inverse of _wrap_idx: [128, L//16] -> [L]"""
    return np.ascontiguousarray(w[:16].T).reshape(-1)[:L].astype(np.int64)


def emulate(inputs):
    """Numpy emulation of the exact device dataflow (bf16 casts included).
    Validates all host-side index/one-hot bookkeeping."""
    per_core, shared, T_pad, alphas = _prep(inputs)
    L = T_pad * P
    f32 = np.float32

    xs = [pc["x_shard"].astype(f32) for pc in per_core]       # [1280, 512]
    for l in range(NLAYERS):
        wr_l = np.concatenate([shared["wr"][:, l, k, :] for k in range(KD)],
                              axis=0).astype(f32)             # [512, 512]
        ws_l = np.concatenate([shared["ws"][:, l, k, :] for k in range(KD)],
                              axis=0).astype(f32)
        b_l = shared["bias"][0, l].astype(f32)
        xq = xs
        # p = xq @ Wr, cast fp8, "AllGather"
        p_full = np.concatenate(
            [(xq[c] @ wr_l).astype(F8).astype(f32) for c in range(NCORES)],
            axis=0)                                           # [10240, 512]
        new_xs = []
        for c in range(NCORES):
            nx = np.zeros((NC_NODES, D), f32)
            for b in range(NBLK):
                idx = _unwrap(
                    per_core[c]["idxe"][:, b * (L // 16):(b + 1) * (L // 16)], L)
                idx = idx.copy()
                idx[:LT * P] += c * NC_NODES       # local tiles: shard-relative
                gath = p_full[idx]                            # [L, 512] fp8 exact
                acc = np.zeros((P, D), f32)
                for t in range(T_pad):
                    oh = per_core[c]["ohot"][
                        :, b * T_pad + t, :].astype(f32)      # [128e, 128d]
                    acc += oh.T @ gath[t * P:(t + 1) * P]
                blk = xs[c][b * P:(b + 1) * P]
                acc += xq[c][b * P:(b + 1) * P] @ ws_l + b_l
                val = (acc.astype(f32) + blk)
                if l < NLAYERS - 1:
                    val = np.maximum(val, 0)
                nx[b * P:(b + 1) * P] = val.astype(BF16).astype(f32)
            new_xs.append(nx)
        xs = new_xs
    # pooling
    pooled_T = np.zeros((D, G), f32)
    for c in range(NCORES):
        goh = per_core[c]["goh"].astype(f32)
        for b in range(NBLK):
            blk = xs[c][b * P:(b + 1) * P].astype(BF16).astype(f32)
            for j in range(KD):
                pooled_T[j * P:(j + 1) * P] += (
                    blk[:, j * P:(j + 1) * P].T @ goh[:, b * G:(b + 1) * G])
    inv = shared["invt"][0, :G].astype(f32)
    pooled_T = (pooled_T * inv[None, :]).astype(BF16).astype(f32)
    wlin = np.concatenate([shared["wlin"][:, k, :] for k in range(KD)],
                          axis=0).astype(f32)                 # [512, 128]
    out_T = wlin.T @ pooled_T + shared["blin"][:, :1]         # [128, 64]
    return np.ascontiguousarray(out_T.T).astype(np.float32)


def _build(T_pad, alphas=None, enable_asserts=False):
    if alphas is None:
        alphas = [1.0] * NLAYERS
    import os
    n_layers = int(os.environ.get("GCN_LAYERS", NLAYERS))
    no_gather = bool(int(os.environ.get("GCN_NO_GATHER", "0")))
    no_cc = bool(int(os.environ.get("GCN_NO_CC", "0")))
    bP, bA, bT = (int(v) for v in os.environ.get("GCN_BANKS", "2,3,2").split(","))
    gbufs = int(os.environ.get("GCN_GBUFS", "3"))
    gsplit = int(os.environ.get("GCN_GSPLIT", "2"))
    no_tr = bool(int(os.environ.get("GCN_NO_TR", "0")))      # timing expts only
    import concourse.bass as bass
    import concourse.mybir as mybir
    import concourse.tile as tile
    from concourse import bacc

    F32 = mybir.dt.float32
    BF = mybir.dt.bfloat16
    FP8 = mybir.dt.float8e4
    I16 = mybir.dt.int16
    DROW = mybir.MatmulPerfMode.DoubleRow
    ADD = mybir.AluOpType.add
    MUL = mybir.AluOpType.mult
    L = T_pad * P
    RG = [list(range(NCORES))]

    nc = bacc.Bacc("TRN2", target_bir_lowering=False, debug=False,
                   enable_asserts=enable_asserts, num_devices=NCORES)

    # per-core inputs
    x_d = nc.dram_tensor("x_shard", [NC_NODES, D], BF, kind="ExternalInput")
    xt0_d = nc.dram_tensor("xt0", [P, KD, NC_NODES], BF, kind="ExternalInput")
    oh_d = nc.dram_tensor("ohot", [P, NBLK * T_pad, P], FP8, kind="ExternalInput")
    idxe_d = nc.dram_tensor("idxe", [P, NBLK * (L // 16)], I16, kind="ExternalInput")
    idxl_d = nc.dram_tensor("idxl", [P, NBLK * LT * 8], I16, kind="ExternalInput")
    goh_d = nc.dram_tensor("goh", [P, NBLK * G], BF, kind="ExternalInput")
    # shared inputs
    wr_d = nc.dram_tensor("wr", [P, NLAYERS, KD, D], BF, kind="ExternalInput")
    ws_d = nc.dram_tensor("ws", [P, NLAYERS, KD, D], BF, kind="ExternalInput")
    bias_d = nc.dram_tensor("bias", [P, NLAYERS, D], BF, kind="ExternalInput")
    ones_d = nc.dram_tensor("ones", [P, P], BF, kind="ExternalInput")
    wlin_d = nc.dram_tensor("wlin", [P, KD, OUT], BF, kind="ExternalInput")
    blin_d = nc.dram_tensor("blin", [OUT, 1], F32, kind="ExternalInput")
    invt_d = nc.dram_tensor("invt", [P, KD * G], F32, kind="ExternalInput")
    ident_d = nc.dram_tensor("ident", [P, P], BF, kind="ExternalInput")
    # internal DRAM (double-buffered by layer parity so the AllGather for
    # layer l+1 never WAR-depends on layer l's gathers)
    p_shard = [nc.dram_tensor(f"p_shard{i}", [NC_NODES, D], FP8) for i in (0, 1)]
    p_full = [nc.dram_tensor(f"p_full{i}", [NPAD, D], FP8, addr_space="Shared")
              for i in (0, 1)]
    pool_in = nc.dram_tensor("pool_in", [P, KD * G], F32)
    pool_out = nc.dram_tensor("pool_out", [P, KD * G], F32, addr_space="Shared")
    # output
    out_d = nc.dram_tensor("out_t", [OUT, G], F32, kind="ExternalOutput")

    with tile.TileContext(nc) as tc:
        with (
            tc.tile_pool(name="const", bufs=1) as const,
            tc.tile_pool(name="xs", bufs=2) as xpool,
            tc.tile_pool(name="xt", bufs=2) as xtpool,
            tc.tile_pool(name="gath", bufs=gbufs) as gpool,
            tc.tile_pool(name="small", bufs=int(os.environ.get("GCN_SBUFS", "4"))) as spool,
            tc.tile_pool(name="psP", bufs=bP, space="PSUM") as psP,
            tc.tile_pool(name="psA", bufs=bA, space="PSUM") as psA,
            tc.tile_pool(name="psS", bufs=1, space="PSUM") as psS,
            tc.tile_pool(name="psT", bufs=bT, space="PSUM") as psT,
        ):
            # ---- constants to SBUF, critical-path-first: the prologue
            # projection needs only xt0 + Wr[0]; everything else streams in
            # behind it so PE starts ~5us in instead of ~37us
            xt_cur = xtpool.tile([P, KD, NC_NODES], BF, tag="xt")
            nc.sync.dma_start(xt_cur[:], xt0_d[:])
            wr_sb = const.tile([P, NLAYERS, KD, D], BF, tag="wr")
            nc.sync.dma_start(wr_sb[:, 0], wr_d[:, 0])
            ws_sb = const.tile([P, NLAYERS, KD, D], BF, tag="ws")
            nc.sync.dma_start(ws_sb[:, 0], ws_d[:, 0])
            bias_sb = const.tile([P, NLAYERS, D], BF, tag="bias")
            nc.sync.dma_start(bias_sb[:], bias_d[:])
            ones_sb = const.tile([P, P], BF, tag="ones")
            nc.sync.dma_start(ones_sb[:], ones_d[:])
            xs_cur = xpool.tile([P, NBLK, D], BF, tag="xs")
            nc.sync.dma_start(xs_cur[:], x_d.ap().rearrange("(b p) d -> p b d", p=P))
            idxe_sb = const.tile([P, NBLK * (L // 16)], I16, tag="idxe")
            oh_sb = const.tile([P, NBLK * T_pad, P], FP8, tag="oh")

            def load_ohidx(cb):
                nc.sync.dma_start(
                    idxe_sb[:, cb * (L // 16):(cb + 1) * (L // 16)],
                    idxe_d[:, cb * (L // 16):(cb + 1) * (L // 16)])
                nc.sync.dma_start(
                    oh_sb[:, cb * T_pad:(cb + 1) * T_pad, :],
                    oh_d[:, cb * T_pad:(cb + 1) * T_pad, :])

            for cb in range(5):
                load_ohidx(cb)
            ident_sb = const.tile([P, P], BF, tag="ident")
            nc.sync.dma_start(ident_sb[:], ident_d[:])
            # later-layer weights + pooling constants load from inside the
            # layer loop (below) so layer 0's stores/gathers aren't queued
            # behind 4.5MB of not-yet-needed constants
            goh_sb = const.tile([P, NBLK * G], BF, tag="goh")
            wlin_sb = const.tile([P, KD, OUT], BF, tag="wlin")
            blin_sb = const.tile([OUT, 1], F32, tag="blin")
            invt_sb = const.tile([P, KD * G], F32, tag="invt")

            def deferred_loads(l_, b_):
                if l_ == 0 and b_ < 5:
                    load_ohidx(b_ + 5)
                if l_ == 0 and b_ in (3, 5):
                    for wl in (1, 2) if b_ == 3 else (3, 4):
                        nc.sync.dma_start(wr_sb[:, wl], wr_d[:, wl])
                        nc.sync.dma_start(ws_sb[:, wl], ws_d[:, wl])
                elif l_ == 2 and b_ == 0:
                    nc.sync.dma_start(goh_sb[:], goh_d[:])
                elif l_ == 3 and b_ == 0:
                    nc.sync.dma_start(wlin_sb[:], wlin_d[:])
                    nc.sync.dma_start(blin_sb[:], blin_d[:])
                    nc.sync.dma_start(invt_sb[:], invt_d[:])

            def emit_p_block(xt_src, layer, m, pbuf):
                """p[l=layer] block m = x_l[block m] @ Wr_l, into p_shard[pbuf]."""
                pps = psP.tile([P, D], F32, tag="pps", name=f"pps_{layer}_{m}")
                for k in range(KD):
                    nc.tensor.matmul(
                        pps[:],
                        lhsT=xt_src[:, k, m * P:(m + 1) * P],
                        rhs=wr_sb[:, layer, k, :],
                        start=(k == 0), stop=(k == KD - 1))
                p_sb = spool.tile([P, D], FP8, tag="psb", name=f"psb_{layer}_{m}")
                nc.scalar.activation(
                    p_sb[:], pps[:], func=mybir.ActivationFunctionType.Copy)
                nc.sync.dma_start(
                    p_shard[pbuf][m * P:(m + 1) * P, :], p_sb[:])

            def emit_ag(pbuf):
                if no_cc:
                    nc.sync.dma_start(
                        p_full[pbuf][:NC_NODES, :], p_shard[pbuf][:])
                else:
                    nc.gpsimd.collective_compute(
                        "AllGather", mybir.AluOpType.bypass, replica_groups=RG,
                        ins=[p_shard[pbuf][:]], outs=[p_full[pbuf][:]])

            def emit_wsbias(l_, xt_src, b_):
                """root-term matmuls for block b_; queued one block ahead so
                PE has dependency-free work while gathers/AllGather drain"""
                aps = psA.tile([P, D], F32, tag="aps")
                for k in range(KD):
                    nc.tensor.matmul(
                        aps[:],
                        lhsT=xt_src[:, k, b_ * P:(b_ + 1) * P],
                        rhs=ws_sb[:, l_, k, :],
                        start=(k == 0), stop=False)
                nc.tensor.matmul(
                    aps[:], lhsT=ones_sb[:], rhs=bias_sb[:, l_, :],
                    start=False, stop=False)
                return aps

            # prologue: projection for layer 0
            for m in range(NBLK):
                emit_p_block(xt_cur, 0, m, 0)
            emit_ag(0)
            wsdepth = int(os.environ.get("GCN_WSDEPTH", "2"))
            pend = [emit_wsbias(0, xt_cur, b_) for b_ in range(wsdepth)]

            pool_acc = const.tile([P, KD * G], F32, tag="pool_acc")
            for l in range(n_layers):
                pbuf = l % 2
                xs_next = xpool.tile([P, NBLK, D], BF, tag="xs")
                last = l == NLAYERS - 1
                if not last:
                    xt_next = xtpool.tile([P, KD, NC_NODES], BF, tag="xt")
                for b in range(NBLK):
                    deferred_loads(l, b)
                    g = gpool.tile([P, T_pad, D], FP8, tag="g")
                    if no_gather:
                        nc.vector.memset(g[:], 0)
                    else:
                        # tiles [0, LT): local-source edges gathered from
                        # p_shard -- no AllGather dependency, so their segment
                        # matmuls overlap the collective. Remaining tiles read
                        # p_full in even-sized chunks (DoubleRow pairs never
                        # span chunks)
                        col0 = b * (L // 16)
                        nc.gpsimd.dma_gather(
                            g[:, 0:LT, :], p_shard[pbuf][:],
                            idxe_sb[:, col0:col0 + LT * 8],
                            LT * P, LT * P, D, single_packet=False)
                        rem = T_pad - LT
                        if gsplit == 2 and rem > 6:
                            # uneven split: small first chunk releases segs
                            # sooner at the same Pool desc-gen call count
                            rf = int(os.environ.get("GCN_RFIRST", "8"))
                            bounds = [LT, LT + rf, T_pad]
                        else:
                            nsp = min(gsplit, max(1, rem // 2))
                            th = -2 * (-rem // (2 * nsp))
                            bounds = list(range(LT, T_pad, th)) + [T_pad]
                        for s0, s1 in zip(bounds[:-1], bounds[1:]):
                            nc.gpsimd.dma_gather(
                                g[:, s0:s1, :], p_full[pbuf][:],
                                idxe_sb[:, col0 + s0 * 8:col0 + s1 * 8],
                                (s1 - s0) * P, (s1 - s0) * P, D,
                                single_packet=False)
                    aps = pend.pop(0)
                    for t in range(0, T_pad, 2):
                        nc.tensor.matmul(
                            aps[:],
                            lhsT=oh_sb[:, b * T_pad + t:b * T_pad + t + 2, :],
                            rhs=g[:, t:t + 2, :],
                            start=False,
                            stop=(t + 2 >= T_pad),
                            perf_mode=DROW)
                    if last:
                        nc.vector.tensor_tensor(
                            xs_next[:, b, :], aps[:], xs_cur[:, b, :], op=ADD)
                        # pooling partials for this block, interleaved so they
                        # hide under later blocks' gathers; accumulate in SBUF
                        # so only one PSUM bank cycles here
                        for j in range(KD):
                            pps2 = psS.tile([P, G], F32, tag="pool")
                            nc.tensor.matmul(
                                pps2[:],
                                lhsT=xs_next[:, b, j * P:(j + 1) * P],
                                rhs=goh_sb[:, b * G:(b + 1) * G],
                                start=True, stop=True)
                            if b == 0:
                                nc.scalar.activation(
                                    pool_acc[:, j * G:(j + 1) * G], pps2[:],
                                    func=mybir.ActivationFunctionType.Copy)
                            else:
                                nc.vector.tensor_tensor(
                                    pool_acc[:, j * G:(j + 1) * G],
                                    pool_acc[:, j * G:(j + 1) * G], pps2[:],
                                    op=ADD)
                    else:
                        t1 = spool.tile([P, D], BF, tag="t1")
                        nc.vector.tensor_tensor(
                            t1[:], aps[:], xs_cur[:, b, :], op=ADD)
                        nc.vector.tensor_scalar_max(
                            xs_next[:, b, :], t1[:], 0.0)
                        # transpose new block into xt_next (channel-major)
                        if no_tr:
                            nc.vector.tensor_copy(
                                xt_next[:, :, b * P:(b + 1) * P],
                                xs_next[:, b, :].rearrange(
                                    "p (j q) -> p j q", j=KD)[:, :, :P])
                        else:
                            for j in range(KD):
                                trps = psT.tile([P, P], BF, tag="tr")
                                nc.tensor.transpose(
                                    trps[:], xs_next[:, b, j * P:(j + 1) * P],
                                    ident_sb[:])
                                nc.vector.tensor_copy(
                                    xt_next[:, j, b * P:(b + 1) * P], trps[:])
                        # pipelined projection for layer l+1, block b
                        emit_p_block(xt_next, l + 1, b, 1 - pbuf)
                    nb = b + wsdepth
                    if nb < NBLK:
                        pend.append(emit_wsbias(l, xt_cur, nb))
                    elif b == NBLK - 1 and not last:
                        # burst the next layer's first root-term groups HERE,
                        # after the last proj: this is the only queue position
                        # whose work can fill the AllGather+gather stall
                        for b_ in range(wsdepth):
                            pend.append(emit_wsbias(l + 1, xt_next, b_))
                if not last:
                    emit_ag(1 - pbuf)
                    xt_cur = xt_next
                xs_cur = xs_next

            # ---- pooling partials were accumulated inside the last layer's
            # block loop (one PSUM bank per 128-channel chunk)
            nc.sync.dma_start(pool_in[:], pool_acc[:])
            if no_cc:
                nc.sync.dma_start(pool_out[:], pool_acc[:])
            else:
                nc.gpsimd.collective_compute(
                    "AllReduce", ADD, replica_groups=RG,
                    ins=[pool_in[:]], outs=[pool_out[:]])
            pool2 = spool.tile([P, KD * G], F32, tag="pool2")
            nc.sync.dma_start(pool2[:], pool_out[:])
            poolbf = spool.tile([P, KD * G], BF, tag="poolbf")
            nc.vector.tensor_tensor(poolbf[:], pool2[:], invt_sb[:], op=MUL)
            fin_ps = psS.tile([P, G], F32, tag="pool", name="fin_ps")
            for k in range(KD):
                nc.tensor.matmul(
                    fin_ps[:], lhsT=wlin_sb[:, k, :],
                    rhs=poolbf[:, k * G:(k + 1) * G],
                    start=(k == 0), stop=(k == KD - 1))
            fin_sb = spool.tile([OUT, G], F32, tag="fin_sb")
            nc.vector.tensor_tensor(
                fin_sb[:], fin_ps[:], blin_sb[:, :1].to_broadcast([OUT, G]),
                op=ADD)
            nc.sync.dma_start(out_d[:], fin_sb[:])

    nc.compile()
    return nc


def kernel(**inputs):
    import os
    from concourse.bass_utils import run_bass_kernel_spmd

    per_core, shared, T_pad, alphas = _prep(inputs)
    nc = _build(T_pad, alphas)
    in_maps = [{**pc, **shared} for pc in per_core]
    trace = bool(int(os.environ.get("GCN_TRACE", "0")))
    res = run_bass_kernel_spmd(nc, in_maps, core_ids=list(range(NCORES)),
                               trace=trace)
    if trace:
        print(f"HW exec time: {res.exec_time_ns} ns")
        if res.instructions_and_trace is not None:
            print("trace:", res.instructions_and_trace[1])
    out_t = res.results[0]["out_t"]
    return np.ascontiguousarray(out_t.T).astype(np.float32)



# revision 41
# speedup vs baseline: 1.0034x; 1.0012x over previous
"""Distributed GCN (5x GraphConv(add) + residual/ReLU + mean-pool + linear)
for 8 Trainium2 NeuronCores.

Sharding: nodes partitioned contiguously across cores (1280 nodes/core, padded
to 10240). Each core owns the edges whose *destination* lands in its shard.
Aggregation is computed as A@(x@Wr): project first (p = x@Wr, bf16 matmuls),
quantize p to fp8-e4m3, AllGather, gather p[src[e]] rows with SWDGE
dma_gather (512B rows, half the bf16 traffic), then reduce edge tiles onto
destination nodes with fp8 DoubleRow segment matmuls (2 edge-tiles per PE
instruction). Weights/x stay bf16: fp8 weight-quant error is correlated
across nodes and survives mean-pooling, while fp8-p error averages out over
~16 aggregated edges (rel err 7e-3 vs 5e-3 all-bf16).

Overlap: the first LT=2 edge-tiles of each block hold only local-source
edges, gathered from p_shard before the AllGather lands; root-term (x@Ws +
bias) PSUM groups are emitted one block ahead (psA ping-pong) so PE has
dependency-free work while gathers drain; constants stream critical-path
first (xt0+Wr[0] before everything); residual add + ReLU both on DVE (no
cross-engine hop on the xt critical path); p PSUM->fp8 casts and mean-pool
accumulation on the Activation engine. fp32 PSUM accumulation throughout.
"""

import numpy as np
import ml_dtypes

BF16 = ml_dtypes.bfloat16
F8 = ml_dtypes.float8_e4m3

N, E, D, OUT, G = 10000, 160000, 512, 128, 64
NCORES, P = 8, 128
NBLK = 10                     # 128-node blocks per core
NC_NODES = NBLK * P           # 1280
NPAD = NCORES * NC_NODES      # 10240
NLAYERS = 5
KD = D // P                   # 4 chunks of in-channels
LT = 2                        # leading edge-tiles per block reserved for
                              # local-source edges (gathered from p_shard
                              # before the AllGather completes)


def _wrap_idx(a):
    """[L] ints -> [128, L//16] int16 SWDGE index layout (16-partition wrap,
    replicated for the 8 Q7 cores)."""
    L = len(a)
    w = a.astype(np.int16).reshape(L // 16, 16).T
    return np.ascontiguousarray(np.tile(w, (8, 1)))


_ALPHA_CACHE = {}


def _layer_alphas(x, ss_, ds_, inputs):
    """Per-layer power-of-2 scales so fp8(alpha_l * x_l) never overflows
    e4m3 (+-240). Exact f32 forward pass with reduceat segment sums."""
    key = (float(x[0, :8].sum()), float(x[-1, :8].sum()))
    if key in _ALPHA_CACHE:
        return _ALPHA_CACHE[key]
    node_starts = np.searchsorted(ds_, np.arange(N + 1))
    cnts = np.diff(node_starts)
    nz = cnts > 0
    starts_nz = node_starts[:-1][nz]
    xs = x.astype(np.float32)
    alphas = []
    for l in range(NLAYERS):
        m = float(np.abs(xs).max())
        alphas.append(min(1.0, 2.0 ** np.floor(np.log2(100.0 / max(m, 1e-9)))))
        gath = xs[ss_]
        agg = np.zeros((N, D), np.float32)
        agg[nz] = np.add.reduceat(gath, starts_nz, axis=0)
        wr_l = np.asarray(inputs[f"Wr{l+1}"], np.float32)
        ws_l = np.asarray(inputs[f"Ws{l+1}"], np.float32)
        b_l = np.asarray(inputs[f"b{l+1}"], np.float32)
        pm = float(np.abs(xs @ wr_l).max())   # per-node p = x@Wr is fp8-cast
        assert pm < 200.0, f"fp8 p overflow risk at layer {l}: {pm}"
        val = xs + agg @ wr_l + b_l + xs @ ws_l
        xs = np.maximum(val, 0) if l < NLAYERS - 1 else val
    _ALPHA_CACHE[key] = alphas
    return alphas


def _prep(inputs):
    x = np.asarray(inputs["x"], np.float32)
    ei = np.asarray(inputs["edge_index"]).astype(np.int64)
    batch = np.asarray(inputs["batch"]).astype(np.int64)
    src, dst = ei[0], ei[1]

    order = np.argsort(dst, kind="stable")
    ds_, ss_ = dst[order], src[order]
    alphas = [1.0] * NLAYERS    # x/weights stay bf16 (fp8 weights fail the
                                # error gate: their quant error is correlated
                                # across nodes so pooling can't average it)
    starts = np.searchsorted(ds_, np.arange(0, NPAD + 1, P))
    counts = np.diff(starts)
    T_pad = max(2, int(np.ceil(counts.max() / P)))
    # remote region (T_pad - LT tiles) must hold every block's non-local edges
    nremote = []
    for gb in range(NPAD // P):
        c_ = gb // NBLK
        sl = ss_[starts[gb]:starts[gb + 1]]
        isloc = (sl >= c_ * NC_NODES) & (sl < (c_ + 1) * NC_NODES)
        nloc = min(int(isloc.sum()), LT * P)
        nremote.append(len(sl) - nloc)
    T_pad = max(T_pad, LT + int(np.ceil(max(nremote) / P)))
    T_pad += T_pad % 2          # even, for fp8 DoubleRow tile pairs
    L = T_pad * P

    xp = np.zeros((NPAD, D), np.float32)
    xp[:N] = x

    counts_g = np.bincount(batch, minlength=G)[:G]
    inv = (1.0 / np.maximum(counts_g, 1.0)).astype(np.float32)

    per_core = []
    for c in range(NCORES):
        idx_blocks = []
        loc_blocks = []
        oh_flat = np.zeros((P, NBLK * T_pad, P), F8)
        goh = np.zeros((P, NBLK * G), BF16)
        for b in range(NBLK):
            gb = c * NBLK + b
            lo = gb * P
            s0, s1 = int(starts[gb]), int(starts[gb + 1])
            n = s1 - s0
            sl = ss_[s0:s1]
            dl = ds_[s0:s1] - lo
            isloc = (sl >= c * NC_NODES) & (sl < (c + 1) * NC_NODES)
            loc = np.where(isloc)[0]
            rem = np.where(~isloc)[0]
            nl = min(len(loc), LT * P)
            rest = np.concatenate([rem, loc[nl:]])
            assert len(rest) <= (T_pad - LT) * P, "remote edge region overflow"
            srcs = np.zeros(L, np.int64)
            srcs[:nl] = sl[loc[:nl]] - c * NC_NODES     # p_shard-relative
            srcs[LT * P:LT * P + len(rest)] = sl[rest]  # p_full (global)
            oh = np.zeros((L, P), F8)
            oh[np.arange(nl), dl[loc[:nl]]] = 1
            oh[LT * P + np.arange(len(rest)), dl[rest]] = 1
            idx_blocks.append(_wrap_idx(srcs))
            loc_blocks.append(_wrap_idx(srcs[:LT * P]))
            oh_flat[:, b * T_pad:(b + 1) * T_pad, :] = (
                oh.reshape(T_pad, P, P).transpose(1, 0, 2))
            nodes = lo + np.arange(P)
            valid = nodes < N
            goh[valid, b * G + batch[nodes[valid]]] = 1

        shard = xp[c * NC_NODES:(c + 1) * NC_NODES].astype(BF16)
        xt0 = np.ascontiguousarray(
            shard.T.reshape(KD, P, NC_NODES).transpose(1, 0, 2))
        per_core.append(dict(
            x_shard=np.ascontiguousarray(shard),
            xt0=xt0,
            ohot=oh_flat,
            idxe=np.ascontiguousarray(np.concatenate(idx_blocks, axis=1)),
            idxl=np.ascontiguousarray(np.concatenate(loc_blocks, axis=1)),
            goh=goh,
        ))

    wr = np.zeros((P, NLAYERS, KD, D), BF16)
    ws = np.zeros((P, NLAYERS, KD, D), BF16)
    bias = np.zeros((P, NLAYERS, D), BF16)
    for l in range(NLAYERS):
        wr[:, l] = np.asarray(inputs[f"Wr{l+1}"], np.float32).reshape(
            KD, P, D).transpose(1, 0, 2).astype(BF16)
        ws[:, l] = np.asarray(inputs[f"Ws{l+1}"], np.float32).reshape(
            KD, P, D).transpose(1, 0, 2).astype(BF16)
        bias[0, l] = np.asarray(inputs[f"b{l+1}"], np.float32).astype(BF16)
    ones_e0 = np.zeros((P, P), BF16)
    ones_e0[0, :] = 1
    wlin = np.ascontiguousarray(
        np.asarray(inputs["Wlin"], np.float32).reshape(KD, P, OUT)
        .transpose(1, 0, 2).astype(BF16))
    blin = np.tile(np.asarray(inputs["blin"], np.float32).reshape(OUT, 1),
                   (1, 1)).astype(np.float32)
    shared = dict(
        wr=wr, ws=ws, bias=bias, ones=ones_e0, wlin=wlin, blin=blin,
        invt=np.ascontiguousarray(np.tile(inv, (P, KD)).astype(np.float32)),
        ident=np.eye(P, dtype=BF16),
    )
    return per_core, shared, T_pad, alphas


def _unwrap(w, L):
    """inverse of _wrap_idx: [128, L//16] -> [L]"""
    return np.ascontiguousarray(w[:16].T).reshape(-1)[:L].astype(np.int64)


def emulate(inputs):
    """Numpy emulation of the exact device dataflow (bf16 casts included).
    Validates all host-side index/one-hot bookkeeping."""
    per_core, shared, T_pad, alphas = _prep(inputs)
    L = T_pad * P
    f32 = np.float32

    xs = [pc["x_shard"].astype(f32) for pc in per_core]       # [1280, 512]
    for l in range(NLAYERS):
        wr_l = np.concatenate([shared["wr"][:, l, k, :] for k in range(KD)],
                              axis=0).astype(f32)             # [512, 512]
        ws_l = np.concatenate([shared["ws"][:, l, k, :] for k in range(KD)],
                              axis=0).astype(f32)
        b_l = shared["bias"][0, l].astype(f32)
        xq = xs
        # p = xq @ Wr, cast fp8, "AllGather"
        p_full = np.concatenate(
            [(xq[c] @ wr_l).astype(F8).astype(f32) for c in range(NCORES)],
            axis=0)                                           # [10240, 512]
        new_xs = []
        for c in range(NCORES):
            nx = np.zeros((NC_NODES, D), f32)
            for b in range(NBLK):
                idx = _unwrap(
                    per_core[c]["idxe"][:, b * (L // 16):(b + 1) * (L // 16)], L)
                idx = idx.copy()
                idx[:LT * P] += c * NC_NODES       # local tiles: shard-relative
                gath = p_full[idx]                            # [L, 512] fp8 exact
                acc = np.zeros((P, D), f32)
                for t in range(T_pad):
                    oh = per_core[c]["ohot"][
                        :, b * T_pad + t, :].astype(f32)      # [128e, 128d]
                    acc += oh.T @ gath[t * P:(t + 1) * P]
                blk = xs[c][b * P:(b + 1) * P]
                acc += xq[c][b * P:(b + 1) * P] @ ws_l + b_l
                val = (acc.astype(f32) + blk)
                if l < NLAYERS - 1:
                    val = np.maximum(val, 0)
                nx[b * P:(b + 1) * P] = val.astype(BF16).astype(f32)
            new_xs.append(nx)
        xs = new_xs
    # pooling
    pooled_T = np.zeros((D, G), f32)
    for c in range(NCORES):
        goh = per_core[c]["goh"].astype(f32)
        for b in range(NBLK):
            blk = xs[c][b * P:(b + 1) * P].astype(BF16).astype(f32)
            for j in range(KD):
                pooled_T[j * P:(j + 1) * P] += (
                    blk[:, j * P:(j + 1) * P].T @ goh[:, b * G:(b + 1) * G])
    inv = shared["invt"][0, :G].astype(f32)
    pooled_T = (pooled_T * inv[None, :]).astype(BF16).astype(f32)
    wlin = np.concatenate([shared["wlin"][:, k, :] for k in range(KD)],
                          axis=0).astype(f32)                 # [512, 128]
    out_T = wlin.T @ pooled_T + shared["blin"][:, :1]         # [128, 64]
    return np.ascontiguousarray(out_T.T).astype(np.float32)


def _build(T_pad, alphas=None, enable_asserts=False):
    if alphas is None:
        alphas = [1.0] * NLAYERS
    import os
    n_layers = int(os.environ.get("GCN_LAYERS", NLAYERS))
    no_gather = bool(int(os.environ.get("GCN_NO_GATHER", "0")))
    no_cc = bool(int(os.environ.get("GCN_NO_CC", "0")))
    bP, bA, bT = (int(v) for v in os.environ.get("GCN_BANKS", "2,3,2").split(","))
    gbufs = int(os.environ.get("GCN_GBUFS", "3"))
    gsplit = int(os.environ.get("GCN_GSPLIT", "2"))
    no_tr = bool(int(os.environ.get("GCN_NO_TR", "0")))      # timing expts only
    import concourse.bass as bass
    import concourse.mybir as mybir
    import concourse.tile as tile
    from concourse import bacc

    F32 = mybir.dt.float32
    BF = mybir.dt.bfloat16
    FP8 = mybir.dt.float8e4
    I16 = mybir.dt.int16
    DROW = mybir.MatmulPerfMode.DoubleRow
    ADD = mybir.AluOpType.add
    MUL = mybir.AluOpType.mult
    L = T_pad * P
    RG = [list(range(NCORES))]

    nc = bacc.Bacc("TRN2", target_bir_lowering=False, debug=False,
                   enable_asserts=enable_asserts, num_devices=NCORES)

    # per-core inputs
    x_d = nc.dram_tensor("x_shard", [NC_NODES, D], BF, kind="ExternalInput")
    xt0_d = nc.dram_tensor("xt0", [P, KD, NC_NODES], BF, kind="ExternalInput")
    oh_d = nc.dram_tensor("ohot", [P, NBLK * T_pad, P], FP8, kind="ExternalInput")
    idxe_d = nc.dram_tensor("idxe", [P, NBLK * (L // 16)], I16, kind="ExternalInput")
    idxl_d = nc.dram_tensor("idxl", [P, NBLK * LT * 8], I16, kind="ExternalInput")
    goh_d = nc.dram_tensor("goh", [P, NBLK * G], BF, kind="ExternalInput")
    # shared inputs
    wr_d = nc.dram_tensor("wr", [P, NLAYERS, KD, D], BF, kind="ExternalInput")
    ws_d = nc.dram_tensor("ws", [P, NLAYERS, KD, D], BF, kind="ExternalInput")
    bias_d = nc.dram_tensor("bias", [P, NLAYERS, D], BF, kind="ExternalInput")
    ones_d = nc.dram_tensor("ones", [P, P], BF, kind="ExternalInput")
    wlin_d = nc.dram_tensor("wlin", [P, KD, OUT], BF, kind="ExternalInput")
    blin_d = nc.dram_tensor("blin", [OUT, 1], F32, kind="ExternalInput")
    invt_d = nc.dram_tensor("invt", [P, KD * G], F32, kind="ExternalInput")
    ident_d = nc.dram_tensor("ident", [P, P], BF, kind="ExternalInput")
    # internal DRAM (double-buffered by layer parity so the AllGather for
    # layer l+1 never WAR-depends on layer l's gathers)
    p_shard = [nc.dram_tensor(f"p_shard{i}", [NC_NODES, D], FP8) for i in (0, 1)]
    p_full = [nc.dram_tensor(f"p_full{i}", [NPAD, D], FP8, addr_space="Shared")
              for i in (0, 1)]
    pool_in = nc.dram_tensor("pool_in", [P, KD * G], F32)
    pool_out = nc.dram_tensor("pool_out", [P, KD * G], F32, addr_space="Shared")
    # output
    out_d = nc.dram_tensor("out_t", [OUT, G], F32, kind="ExternalOutput")

    with tile.TileContext(nc) as tc:
        with (
            tc.tile_pool(name="const", bufs=1) as const,
            tc.tile_pool(name="xs", bufs=2) as xpool,
            tc.tile_pool(name="xt", bufs=2) as xtpool,
            tc.tile_pool(name="gath", bufs=gbufs) as gpool,
            tc.tile_pool(name="small", bufs=int(os.environ.get("GCN_SBUFS", "4"))) as spool,
            tc.tile_pool(name="psP", bufs=bP, space="PSUM") as psP,
            tc.tile_pool(name="psA", bufs=bA, space="PSUM") as psA,
            tc.tile_pool(name="psS", bufs=1, space="PSUM") as psS,
            tc.tile_pool(name="psT", bufs=bT, space="PSUM") as psT,
        ):
            # ---- constants to SBUF, critical-path-first: the prologue
            # projection needs only xt0 + Wr[0]; everything else streams in
            # behind it so PE starts ~5us in instead of ~37us
            xt_cur = xtpool.tile([P, KD, NC_NODES], BF, tag="xt")
            nc.sync.dma_start(xt_cur[:], xt0_d[:])
            wr_sb = const.tile([P, NLAYERS, KD, D], BF, tag="wr")
            nc.sync.dma_start(wr_sb[:, 0], wr_d[:, 0])
            ws_sb = const.tile([P, NLAYERS, KD, D], BF, tag="ws")
            nc.sync.dma_start(ws_sb[:, 0], ws_d[:, 0])
            bias_sb = const.tile([P, NLAYERS, D], BF, tag="bias")
            nc.sync.dma_start(bias_sb[:], bias_d[:])
            ones_sb = const.tile([P, P], BF, tag="ones")
            nc.sync.dma_start(ones_sb[:], ones_d[:])
            xs_cur = xpool.tile([P, NBLK, D], BF, tag="xs")
            nc.sync.dma_start(xs_cur[:], x_d.ap().rearrange("(b p) d -> p b d", p=P))
            idxe_sb = const.tile([P, NBLK * (L // 16)], I16, tag="idxe")
            oh_sb = const.tile([P, NBLK * T_pad, P], FP8, tag="oh")

            def load_ohidx(cb):
                nc.sync.dma_start(
                    idxe_sb[:, cb * (L // 16):(cb + 1) * (L // 16)],
                    idxe_d[:, cb * (L // 16):(cb + 1) * (L // 16)])
                nc.sync.dma_start(
                    oh_sb[:, cb * T_pad:(cb + 1) * T_pad, :],
                    oh_d[:, cb * T_pad:(cb + 1) * T_pad, :])

            for cb in range(5):
                load_ohidx(cb)
            ident_sb = const.tile([P, P], BF, tag="ident")
            nc.sync.dma_start(ident_sb[:], ident_d[:])
            # later-layer weights + pooling constants load from inside the
            # layer loop (below) so layer 0's stores/gathers aren't queued
            # behind 4.5MB of not-yet-needed constants
            goh_sb = const.tile([P, NBLK * G], BF, tag="goh")
            wlin_sb = const.tile([P, KD, OUT], BF, tag="wlin")
            blin_sb = const.tile([OUT, 1], F32, tag="blin")
            invt_sb = const.tile([P, KD * G], F32, tag="invt")

            def deferred_loads(l_, b_):
                if l_ == 0 and b_ < 5:
                    load_ohidx(b_ + 5)
                if l_ == 0 and b_ in (2, 4):
                    for wl in (1, 2) if b_ == 2 else (3, 4):
                        nc.sync.dma_start(wr_sb[:, wl], wr_d[:, wl])
                        nc.sync.dma_start(ws_sb[:, wl], ws_d[:, wl])
                elif l_ == 2 and b_ == 0:
                    nc.sync.dma_start(goh_sb[:], goh_d[:])
                elif l_ == 3 and b_ == 0:
                    nc.sync.dma_start(wlin_sb[:], wlin_d[:])
                    nc.sync.dma_start(blin_sb[:], blin_d[:])
                    nc.sync.dma_start(invt_sb[:], invt_d[:])

            def emit_p_block(xt_src, layer, m, pbuf):
                """p[l=layer] block m = x_l[block m] @ Wr_l, into p_shard[pbuf]."""
                pps = psP.tile([P, D], F32, tag="pps", name=f"pps_{layer}_{m}")
                for k in range(KD):
                    nc.tensor.matmul(
                        pps[:],
                        lhsT=xt_src[:, k, m * P:(m + 1) * P],
                        rhs=wr_sb[:, layer, k, :],
                        start=(k == 0), stop=(k == KD - 1))
                p_sb = spool.tile([P, D], FP8, tag="psb", name=f"psb_{layer}_{m}")
                nc.scalar.activation(
                    p_sb[:], pps[:], func=mybir.ActivationFunctionType.Copy)
                nc.sync.dma_start(
                    p_shard[pbuf][m * P:(m + 1) * P, :], p_sb[:])

            def emit_ag(pbuf):
                if no_cc:
                    nc.sync.dma_start(
                        p_full[pbuf][:NC_NODES, :], p_shard[pbuf][:])
                else:
                    nc.gpsimd.collective_compute(
                        "AllGather", mybir.AluOpType.bypass, replica_groups=RG,
                        ins=[p_shard[pbuf][:]], outs=[p_full[pbuf][:]])

            def emit_wsbias(l_, xt_src, b_):
                """root-term matmuls for block b_; queued one block ahead so
                PE has dependency-free work while gathers/AllGather drain"""
                aps = psA.tile([P, D], F32, tag="aps")
                for k in range(KD):
                    nc.tensor.matmul(
                        aps[:],
                        lhsT=xt_src[:, k, b_ * P:(b_ + 1) * P],
                        rhs=ws_sb[:, l_, k, :],
                        start=(k == 0), stop=False)
                nc.tensor.matmul(
                    aps[:], lhsT=ones_sb[:], rhs=bias_sb[:, l_, :],
                    start=False, stop=False)
                return aps

            # prologue: projection for layer 0
            for m in range(NBLK):
                emit_p_block(xt_cur, 0, m, 0)
            emit_ag(0)
            wsdepth = int(os.environ.get("GCN_WSDEPTH", "2"))
            pend = [emit_wsbias(0, xt_cur, b_) for b_ in range(wsdepth)]

            pool_acc = const.tile([P, KD * G], F32, tag="pool_acc")
            for l in range(n_layers):
                pbuf = l % 2
                xs_next = xpool.tile([P, NBLK, D], BF, tag="xs")
                last = l == NLAYERS - 1
                if not last:
                    xt_next = xtpool.tile([P, KD, NC_NODES], BF, tag="xt")
                for b in range(NBLK):
                    deferred_loads(l, b)
                    g = gpool.tile([P, T_pad, D], FP8, tag="g")
                    if no_gather:
                        nc.vector.memset(g[:], 0)
                    else:
                        # tiles [0, LT): local-source edges gathered from
                        # p_shard -- no AllGather dependency, so their segment
                        # matmuls overlap the collective. Remaining tiles read
                        # p_full in even-sized chunks (DoubleRow pairs never
                        # span chunks)
                        col0 = b * (L // 16)
                        nc.gpsimd.dma_gather(
                            g[:, 0:LT, :], p_shard[pbuf][:],
                            idxe_sb[:, col0:col0 + LT * 8],
                            LT * P, LT * P, D, single_packet=False)
                        rem = T_pad - LT
                        if gsplit == 2 and rem > 6:
                            # uneven split: small first chunk releases segs
                            # sooner at the same Pool desc-gen call count
                            rf = int(os.environ.get("GCN_RFIRST", "8"))
                            bounds = [LT, LT + rf, T_pad]
                        else:
                            nsp = min(gsplit, max(1, rem // 2))
                            th = -2 * (-rem // (2 * nsp))
                            bounds = list(range(LT, T_pad, th)) + [T_pad]
                        for s0, s1 in zip(bounds[:-1], bounds[1:]):
                            nc.gpsimd.dma_gather(
                                g[:, s0:s1, :], p_full[pbuf][:],
                                idxe_sb[:, col0 + s0 * 8:col0 + s1 * 8],
                                (s1 - s0) * P, (s1 - s0) * P, D,
                                single_packet=False)
                    aps = pend.pop(0)
                    for t in range(0, T_pad, 2):
                        nc.tensor.matmul(
                            aps[:],
                            lhsT=oh_sb[:, b * T_pad + t:b * T_pad + t + 2, :],
                            rhs=g[:, t:t + 2, :],
                            start=False,
                            stop=(t + 2 >= T_pad),
                            perf_mode=DROW)
                    if last:
                        nc.vector.tensor_tensor(
                            xs_next[:, b, :], aps[:], xs_cur[:, b, :], op=ADD)
                        # pooling partials for this block, interleaved so they
                        # hide under later blocks' gathers; accumulate in SBUF
                        # so only one PSUM bank cycles here
                        for j in range(KD):
                            pps2 = psS.tile([P, G], F32, tag="pool")
                            nc.tensor.matmul(
                                pps2[:],
                                lhsT=xs_next[:, b, j * P:(j + 1) * P],
                                rhs=goh_sb[:, b * G:(b + 1) * G],
                                start=True, stop=True)
                            if b == 0:
                                nc.scalar.activation(
                                    pool_acc[:, j * G:(j + 1) * G], pps2[:],
                                    func=mybir.ActivationFunctionType.Copy)
                            else:
                                nc.vector.tensor_tensor(
                                    pool_acc[:, j * G:(j + 1) * G],
                                    pool_acc[:, j * G:(j + 1) * G], pps2[:],
                                    op=ADD)
                    else:
                        t1 = spool.tile([P, D], BF, tag="t1")
                        nc.vector.tensor_tensor(
                            t1[:], aps[:], xs_cur[:, b, :], op=ADD)
                        nc.vector.tensor_scalar_max(
                            xs_next[:, b, :], t1[:], 0.0)
                        # transpose new block into xt_next (channel-major)
                        if no_tr:
                            nc.vector.tensor_copy(
                                xt_next[:, :, b * P:(b + 1) * P],
                                xs_next[:, b, :].rearrange(
                                    "p (j q) -> p j q", j=KD)[:, :, :P])
                        else:
                            for j in range(KD):
                                trps = psT.tile([P, P], BF, tag="tr")
                                nc.tensor.transpose(
                                    trps[:], xs_next[:, b, j * P:(j + 1) * P],
                                    ident_sb[:])
                                nc.vector.tensor_copy(
                                    xt_next[:, j, b * P:(b + 1) * P], trps[:])
                        # pipelined projection for layer l+1, block b
                        emit_p_block(xt_next, l + 1, b, 1 - pbuf)
                    nb = b + wsdepth
                    if nb < NBLK:
                        pend.append(emit_wsbias(l, xt_cur, nb))
                    elif b == NBLK - 1 and not last:
                        # burst the next layer's first root-term groups HERE,
                        # after the last proj: this is the only queue position
                        # whose work can fill the AllGather+gather stall
                        for b_ in range(wsdepth):
                            pend.append(emit_wsbias(l + 1, xt_next, b_))
                if not last:
                    emit_ag(1 - pbuf)
                    xt_cur = xt_next
                xs_cur = xs_next

            # ---- pooling partials were accumulated inside the last layer's
            # block loop (one PSUM bank per 128-channel chunk)
            nc.sync.dma_start(pool_in[:], pool_acc[:])
            if no_cc:
                nc.sync.dma_start(pool_out[:], pool_acc[:])
            else:
                nc.gpsimd.collective_compute(
                    "AllReduce", ADD, replica_groups=RG,
                    ins=[pool_in[:]], outs=[pool_out[:]])
            pool2 = spool.tile([P, KD * G], F32, tag="pool2")
            nc.sync.dma_start(pool2[:], pool_out[:])
            poolbf = spool.tile([P, KD * G], BF, tag="poolbf")
            nc.vector.tensor_tensor(poolbf[:], pool2[:], invt_sb[:], op=MUL)
            fin_ps = psS.tile([P, G], F32, tag="pool", name="fin_ps")
            for k in range(KD):
                nc.tensor.matmul(
                    fin_ps[:], lhsT=wlin_sb[:, k, :],
                    rhs=poolbf[:, k * G:(k + 1) * G],
                    start=(k == 0), stop=(k == KD - 1))
            fin_sb = spool.tile([OUT, G], F32, tag="fin_sb")
            nc.vector.tensor_tensor(
                fin_sb[:], fin_ps[:], blin_sb[:, :1].to_broadcast([OUT, G]),
                op=ADD)
            nc.sync.dma_start(out_d[:], fin_sb[:])

    nc.compile()
    return nc


def kernel(**inputs):
    import os
    from concourse.bass_utils import run_bass_kernel_spmd

    per_core, shared, T_pad, alphas = _prep(inputs)
    nc = _build(T_pad, alphas)
    in_maps = [{**pc, **shared} for pc in per_core]
    trace = bool(int(os.environ.get("GCN_TRACE", "0")))
    res = run_bass_kernel_spmd(nc, in_maps, core_ids=list(range(NCORES)),
                               trace=trace)
    if trace:
        print(f"HW exec time: {res.exec_time_ns} ns")
        if res.instructions_and_trace is not None:
            print("trace:", res.instructions_and_trace[1])
    out_t = res.results[0]["out_t"]
    return np.ascontiguousarray(out_t.T).astype(np.float32)

